# revision 26
# baseline (speedup 1.0000x reference)
"""Multi-head attention (B=4, S=2048, E=1024, H=16) on 8 TRN2 NeuronCores.

Sharding: tensor-parallel over heads. Core c computes output columns
[128c, 128c+128) (heads 2c and 2c+1). Inputs q,v are fed to every core
pre-transposed to [B, E, S] so projection matmuls can use the token dim
as the moving (N=512) operand; W*/b* are column-sliced per core.

On-chip layout (per core, per batch):
  qpT/kpT/vpT [128 (= 2 heads x 64 d), 2048 tok]  -- projections, transposed
  scoresT [k_tok, q_tok] so exp+AV contraction keeps k on partitions
  softmax sums come from a ones-column appended to token-major v (M=66
  augmented AV matmul; col 65 is zero padding for the fp32r even-width
  rule); no max subtraction (scores ~ N(0, 0.25^2)).

All matmuls run in float32r (TF32-like, full PE rate at N>=512). fp32r
operands must be produced as fp32r, so matmul-feeding tiles are fp32r
and host inputs are pre-rounded.
"""

import numpy as np
from contextlib import ExitStack

import concourse.bass as bass
import concourse.tile as tile
from concourse import bacc, mybir
from concourse.bass_utils import run_bass_kernel_spmd

B, SQ, SK, E, H = 4, 2048, 2048, 1024, 16
NCORES = 8
CPC = E // NCORES          # output cols per core = 128
D = E // H                 # head dim = 64
NE = E // 128              # contraction tiles for projections = 8
NKT = SK // 128            # k-token tiles = 16
NQB = SQ // 512            # q blocks of 512 = 4
SCALE = 1.0 / np.sqrt(E)   # faithful to reference: 1/sqrt(embed_dim)

F32 = mybir.dt.float32
F32R = mybir.dt.float32r
BF16 = mybir.dt.bfloat16
EXP = mybir.ActivationFunctionType.Exp


def _body(ctx: ExitStack, tc: "tile.TileContext", out, qT, vT, wq, wk, wv,
          bq, bk, bv, ident, aug_const, zeros):
    nc = tc.nc

    const = ctx.enter_context(tc.tile_pool(name="const", bufs=1))
    stream = ctx.enter_context(tc.tile_pool(name="stream", bufs=16))
    qp_pool = ctx.enter_context(tc.tile_pool(name="qp", bufs=2))
    kp_pool = ctx.enter_context(tc.tile_pool(name="kp", bufs=2))
    kz0_pool = ctx.enter_context(tc.tile_pool(name="kz0", bufs=2))
    kz1_pool = ctx.enter_context(tc.tile_pool(name="kz1", bufs=2))
    vp_pool = ctx.enter_context(tc.tile_pool(name="vp", bufs=2))
    aug_pool = ctx.enter_context(tc.tile_pool(name="aug", bufs=20))
    exp_pool = ctx.enter_context(tc.tile_pool(name="exp", bufs=5))
    avs_pool = ctx.enter_context(tc.tile_pool(name="avs", bufs=4))
    rec_pool = ctx.enter_context(tc.tile_pool(name="rec", bufs=4))
    out_pool = ctx.enter_context(tc.tile_pool(name="outp", bufs=6))
    psum_sc = ctx.enter_context(tc.tile_pool(name="psc", bufs=2, space="PSUM"))
    psum_av = ctx.enter_context(tc.tile_pool(name="pav", bufs=2, space="PSUM"))
    psum_pp = ctx.enter_context(tc.tile_pool(name="ppp", bufs=2, space="PSUM"))

    # --- constants: weight tiles [128 (E-slice), 128 (out col)], biases, identity
    w_sb = {}
    for pname, wdram, wdt in (("k", wk, F32R), ("v", wv, F32R),
                              ("q", wq, BF16)):
        for e in range(NE):
            t = const.tile([128, CPC], wdt, tag=f"w{pname}{e}")
            eng = nc.sync if pname == "k" else nc.gpsimd
            eng.dma_start(t[:], wdram[e * 128:(e + 1) * 128, :])
            w_sb[(pname, e)] = t
    b_sb = {}
    for pname, bdram in (("q", bq), ("k", bk), ("v", bv)):
        t = const.tile([CPC, 1], F32, tag=f"b{pname}")
        nc.gpsimd.dma_start(t[:], bdram[:, :])
        b_sb[pname] = t
    id_sb = const.tile([128, 128], F32, tag="ident")
    nc.gpsimd.dma_start(id_sb[:], ident[:, :])
    # [1, 0] per partition, fp32r (memset cannot produce fp32r)
    ones2_sb = const.tile([128, 2], F32R, tag="ones2")
    nc.gpsimd.dma_start(ones2_sb[:], aug_const[:, :])

    pending_norm = []
    for b in range(B):
        # ---------------- projections ----------------
        qpT = qp_pool.tile([CPC, SQ], BF16)
        vpT = vp_pool.tile([CPC, SK], F32)
        # zero-padded per-head K copies: K=128 matmuls run at full PE rate
        # while K=64 runs at half rate. kz0 = [kh0; 0], kz1 = [0; kh1].
        kpT_z0 = kz0_pool.tile([CPC, SK], BF16)
        kpT_z1 = kz1_pool.tile([CPC, SK], BF16)
        nc.gpsimd.dma_start(kpT_z0[64:128, :], zeros[:, :])
        nc.gpsimd.dma_start(kpT_z1[0:64, :], zeros[:, :])
        for jp in range(2):  # 1024-token block pairs (4KB DMA descriptors)
            jps = slice(jp * 1024, (jp + 1) * 1024)
            # v feeds both k- and v-projections
            vtiles = []
            for e in range(NE):
                t = stream.tile([128, 1024], F32R, tag="in", name=f"vin{e}")
                nc.sync.dma_start(t[:], vT[b, e * 128:(e + 1) * 128, jps])
                vtiles.append(t)
            for pname, dst in (("k", None), ("v", vpT)):
                for j2 in range(2):
                    js = slice(jp * 1024 + j2 * 512, jp * 1024 + j2 * 512 + 512)
                    rs = slice(j2 * 512, (j2 + 1) * 512)
                    pp = psum_pp.tile([128, 512], F32, tag="pp")
                    for e in range(NE):
                        nc.tensor.matmul(pp[:], w_sb[(pname, e)][:],
                                         vtiles[e][:, rs],
                                         start=(e == 0), stop=(e == NE - 1))
                    if pname == "k":
                        nc.vector.tensor_scalar_add(kpT_z0[0:64, js],
                                                    pp[0:64, :],
                                                    b_sb["k"][0:64, :])
                        nc.vector.tensor_scalar_add(kpT_z1[64:128, js],
                                                    pp[64:128, :],
                                                    b_sb["k"][64:128, :])
                    else:
                        nc.vector.tensor_scalar_add(dst[:, js], pp[:],
                                                    b_sb[pname][:])
            qtiles = []
            for e in range(NE):
                t = stream.tile([128, 1024], BF16, tag="inq", name=f"qin{e}")
                nc.sync.dma_start(t[:], qT[b, e * 128:(e + 1) * 128, jps])
                qtiles.append(t)
            for j2 in range(2):
                js = slice(jp * 1024 + j2 * 512, jp * 1024 + j2 * 512 + 512)
                rs = slice(j2 * 512, (j2 + 1) * 512)
                pp = psum_pp.tile([128, 512], F32, tag="pp")
                for e in range(NE):
                    nc.tensor.matmul(pp[:], w_sb[("q", e)][:],
                                     qtiles[e][:, rs],
                                     start=(e == 0), stop=(e == NE - 1))
                nc.vector.tensor_scalar_add(qpT[:, js], pp[:], b_sb["q"][:])

        # ---------------- v -> token-major with ones column ----------------
        # vh_aug[t]: [128 k-tok, 132]:
        #   [h0 d(0:64) | 1s | 0pad | h1 d(66:130) | 1s | 0pad]
        vh_aug = []
        for t in range(NKT):
            tp = psum_pp.tile([128, 128], F32, tag="pp")
            nc.tensor.transpose(tp[:], vpT[:, t * 128:(t + 1) * 128], id_sb[:])
            a = aug_pool.tile([128, 132], F32R, tag="aug")
            nc.vector.tensor_copy(a[:, 0:64], tp[:, 0:64])
            nc.vector.tensor_copy(a[:, 66:130], tp[:, 64:128])
            nc.vector.tensor_copy(a[:, 64:66], ones2_sb[:])
            nc.vector.tensor_copy(a[:, 130:132], ones2_sb[:])
            vh_aug.append(a)

        # ---------------- attention ----------------
        # the (PE-heavy) normalize of each qb is deferred into the next
        # qb's kt loop, where the PE has per-kt slack while ACT runs exp
        for qb in range(NQB):
            qs = slice(qb * 512, (qb + 1) * 512)
            av0 = psum_av.tile([66, 512], F32, tag="av")
            av1 = psum_av.tile([66, 512], F32, tag="av")
            for kt in range(NKT):
                ks = slice(kt * 128, (kt + 1) * 128)
                ps = psum_sc.tile([128, 1024], F32)
                # K=128 zero-padded per-head scores (full PE rate)
                nc.tensor.matmul(ps[:, 0:512], kpT_z0[:, ks],
                                 qpT[:, qs], start=True, stop=True)
                nc.tensor.matmul(ps[:, 512:1024], kpT_z1[:, ks],
                                 qpT[:, qs], start=True, stop=True)
                et = exp_pool.tile([128, 1024], F32R)
                nc.scalar.activation(et[:], ps[:], EXP, scale=SCALE)
                nc.tensor.matmul(av0[:], vh_aug[kt][:, 0:66],
                                 et[:, 0:512],
                                 start=(kt == 0), stop=(kt == NKT - 1))
                nc.tensor.matmul(av1[:], vh_aug[kt][:, 66:132],
                                 et[:, 512:1024],
                                 start=(kt == 0), stop=(kt == NKT - 1))
                if kt >= 4 and pending_norm and (kt - 4) < len(pending_norm):
                    pending_norm[kt - 4]()
                    if kt - 4 == len(pending_norm) - 1:
                        pending_norm = []
            # evict av psum now (frees banks); defer transposes/output
            avss = []
            for h, av in ((0, av0), (1, av1)):
                avs = avs_pool.tile([66, 512], F32, tag="avs")
                nc.vector.tensor_copy(avs[:], av[:])
                avss.append(avs)
            pending_norm = _make_norm(nc, tc, out_pool, psum_pp, rec_pool,
                                      out, id_sb, avss, b, qb)
        for fn in pending_norm:
            fn()
        pending_norm = []


def _make_norm(nc, tc, out_pool, psum_pp, rec_pool, out, id_sb, avss, b, qb):
    """Build deferred per-(h,t) normalize+store steps for one qb."""
    ots = [out_pool.tile([128, 128], mybir.dt.float32, tag="ot",
                         name=f"ot{t}") for t in range(4)]
    steps = []

    def make_step(h, t):
        def step():
            avs = avss[h]
            tp = psum_pp.tile([128, 66], mybir.dt.float32, tag="pp",
                              name="tpn")
            nc.tensor.transpose(tp[:], avs[:, t * 128:(t + 1) * 128],
                                id_sb[0:66, 0:66])
            rec = rec_pool.tile([128, 1], mybir.dt.float32, tag="rec",
                                name="rec")
            nc.vector.reciprocal(rec[:], tp[:, 64:65])
            nc.vector.tensor_scalar_mul(
                ots[t][:, h * 64:(h + 1) * 64], tp[:, 0:64], rec[:])
            if h == 1:
                r0 = qb * 512 + t * 128
                nc.sync.dma_start(out[b, r0:r0 + 128, :], ots[t][:])
        return step

    for t in range(4):
        for h in (0, 1):
            steps.append(make_step(h, t))
    # merge per-(h,t) into per-slot pairs so 8 slots fit in 12 kt slots
    return [(lambda a=steps[i], c=steps[i + 1]: (a(), c()))
            for i in range(0, 8, 2)]


_CACHE = {}


def _build():
    if "nc" in _CACHE:
        return _CACHE["nc"]
    nc = bacc.Bacc("TRN2", target_bir_lowering=False, debug=False,
                   enable_asserts=False)
    qT = nc.dram_tensor("qT", [B, E, SQ], BF16, kind="ExternalInput").ap()
    vT = nc.dram_tensor("vT", [B, E, SK], F32R, kind="ExternalInput").ap()
    wq = nc.dram_tensor("wq", [E, CPC], BF16, kind="ExternalInput").ap()
    wk = nc.dram_tensor("wk", [E, CPC], F32R, kind="ExternalInput").ap()
    wv = nc.dram_tensor("wv", [E, CPC], F32R, kind="ExternalInput").ap()
    bq = nc.dram_tensor("bq", [CPC, 1], F32, kind="ExternalInput").ap()
    bk = nc.dram_tensor("bk", [CPC, 1], F32, kind="ExternalInput").ap()
    bv = nc.dram_tensor("bv", [CPC, 1], F32, kind="ExternalInput").ap()
    ident = nc.dram_tensor("ident", [128, 128], F32, kind="ExternalInput").ap()
    aug_const = nc.dram_tensor("aug_const", [128, 2], F32R,
                               kind="ExternalInput").ap()
    zeros = nc.dram_tensor("zeros", [64, SK], BF16,
                           kind="ExternalInput").ap()
    out = nc.dram_tensor("out", [B, SQ, CPC], F32, kind="ExternalOutput").ap()
    with tile.TileContext(nc) as tc:
        with ExitStack() as ctx:
            _body(ctx, tc, out, qT, vT, wq, wk, wv, bq, bk, bv, ident, aug_const,
                  zeros)
    nc.compile()
    _CACHE["nc"] = nc
    return nc


def _round_tf32(x):
    """Round-to-nearest-even to 10 explicit mantissa bits (TF32)."""
    u = np.ascontiguousarray(x, np.float32).view(np.uint32)
    r = (u + 0x1000 + ((u >> 13) & 1)) & np.uint32(0xFFFFE000)
    return r.view(np.float32)


def _in_maps(q, v, Wq, bq, Wk, bk, Wv, bv):
    f = np.float32
    import ml_dtypes
    qT = np.ascontiguousarray(
        np.transpose(np.asarray(q, f), (0, 2, 1))).astype(ml_dtypes.bfloat16)
    vT = _round_tf32(np.transpose(np.asarray(v, f), (0, 2, 1)))
    ident = np.eye(128, dtype=f)
    aug2 = np.tile(np.array([1.0, 0.0], f), (128, 1))
    zeros64 = np.zeros((64, SK), ml_dtypes.bfloat16)
    Wq, Wk, Wv = (np.asarray(x, f) for x in (Wq, Wk, Wv))
    bq, bk, bv = (np.asarray(x, f) for x in (bq, bk, bv))
    maps = []
    for c in range(NCORES):
        sl = slice(c * CPC, (c + 1) * CPC)
        maps.append({
            "qT": qT, "vT": vT, "ident": ident, "aug_const": aug2,
            "zeros": zeros64,
            "wq": np.ascontiguousarray(Wq[:, sl]).astype(ml_dtypes.bfloat16),
            "wk": _round_tf32(Wk[:, sl]),
            "wv": _round_tf32(Wv[:, sl]),
            "bq": np.ascontiguousarray(bq[sl]).reshape(CPC, 1),
            "bk": np.ascontiguousarray(bk[sl]).reshape(CPC, 1),
            "bv": np.ascontiguousarray(bv[sl]).reshape(CPC, 1),
        })
    return maps


def run(trace=False, **inputs):
    nc = _build()
    maps = _in_maps(**inputs)
    res = run_bass_kernel_spmd(nc, maps, core_ids=list(range(NCORES)),
                               trace=trace)
    full = np.concatenate([res.results[c]["out"] for c in range(NCORES)],
                          axis=2)
    return full, res


def kernel(q, v, Wq, bq, Wk, bk, Wv, bv):
    full, _ = run(q=q, v=v, Wq=Wq, bq=bq, Wk=Wk, bk=bk, Wv=Wv, bv=bv)
    return full


# revision 27
# speedup vs baseline: 1.1428x; 1.1428x over previous
"""Multi-head attention (B=4, S=2048, E=1024, H=16) on 8 TRN2 NeuronCores.

Sharding: tensor-parallel over heads. Core c computes output columns
[128c, 128c+128) (heads 2c and 2c+1). Inputs q,v are fed to every core
pre-transposed to [B, E, S] so projection matmuls can use the token dim
as the moving (N=512) operand; W*/b* are column-sliced per core.

On-chip layout (per core, per batch):
  qpT/kpT/vpT [128 (= 2 heads x 64 d), 2048 tok]  -- projections, transposed
  scoresT [k_tok, q_tok] so exp+AV contraction keeps k on partitions
  softmax sums come from a ones-column appended to token-major v (M=66
  augmented AV matmul; col 65 is zero padding for the fp32r even-width
  rule); no max subtraction (scores ~ N(0, 0.25^2)).

All matmuls run in float32r (TF32-like, full PE rate at N>=512). fp32r
operands must be produced as fp32r, so matmul-feeding tiles are fp32r
and host inputs are pre-rounded.
"""

import numpy as np
from contextlib import ExitStack

import concourse.bass as bass
import concourse.tile as tile
from concourse import bacc, mybir
from concourse.bass_utils import run_bass_kernel_spmd

B, SQ, SK, E, H = 4, 2048, 2048, 1024, 16
NCORES = 8
CPC = E // NCORES          # output cols per core = 128
D = E // H                 # head dim = 64
NE = E // 128              # contraction tiles for projections = 8
NKT = SK // 128            # k-token tiles = 16
NQB = SQ // 512            # q blocks of 512 = 4
SCALE = 1.0 / np.sqrt(E)   # faithful to reference: 1/sqrt(embed_dim)

F32 = mybir.dt.float32
F32R = mybir.dt.float32r
BF16 = mybir.dt.bfloat16
EXP = mybir.ActivationFunctionType.Exp


def _body(ctx: ExitStack, tc: "tile.TileContext", out, qT, vT, wq, wk, wv,
          bq, bk, bv, ident, aug_const, zeros):
    nc = tc.nc

    const = ctx.enter_context(tc.tile_pool(name="const", bufs=1))
    stream = ctx.enter_context(tc.tile_pool(name="stream", bufs=16))
    qp_pool = ctx.enter_context(tc.tile_pool(name="qp", bufs=2))
    kp_pool = ctx.enter_context(tc.tile_pool(name="kp", bufs=2))
    kz0_pool = ctx.enter_context(tc.tile_pool(name="kz0", bufs=2))
    kz1_pool = ctx.enter_context(tc.tile_pool(name="kz1", bufs=2))
    vp_pool = ctx.enter_context(tc.tile_pool(name="vp", bufs=2))
    aug_pool = ctx.enter_context(tc.tile_pool(name="aug", bufs=20))
    exp_pool = ctx.enter_context(tc.tile_pool(name="exp", bufs=5))
    avs_pool = ctx.enter_context(tc.tile_pool(name="avs", bufs=4))
    rec_pool = ctx.enter_context(tc.tile_pool(name="rec", bufs=4))
    out_pool = ctx.enter_context(tc.tile_pool(name="outp", bufs=6))
    psum_sc = ctx.enter_context(tc.tile_pool(name="psc", bufs=2, space="PSUM"))
    psum_av = ctx.enter_context(tc.tile_pool(name="pav", bufs=2, space="PSUM"))
    psum_pp = ctx.enter_context(tc.tile_pool(name="ppp", bufs=2, space="PSUM"))

    # --- constants: weight tiles [128 (E-slice), 128 (out col)], biases, identity
    w_sb = {}
    for pname, wdram, wdt in (("q", wq, BF16), ("k", wk, F32R),
                              ("v", wv, F32R)):
        for e in range(NE):
            t = const.tile([128, CPC], wdt, tag=f"w{pname}{e}")
            nc.sync.dma_start(t[:], wdram[e * 128:(e + 1) * 128, :])
            w_sb[(pname, e)] = t
    b_sb = {}
    for pname, bdram in (("q", bq), ("k", bk), ("v", bv)):
        t = const.tile([CPC, 1], F32, tag=f"b{pname}")
        nc.sync.dma_start(t[:], bdram[:, :])
        b_sb[pname] = t
    id_sb = const.tile([128, 128], F32, tag="ident")
    nc.sync.dma_start(id_sb[:], ident[:, :])
    # [1, 0] per partition, fp32r (memset cannot produce fp32r)
    ones2_sb = const.tile([128, 2], F32R, tag="ones2")
    nc.sync.dma_start(ones2_sb[:], aug_const[:, :])

    pending_norm = []
    for b in range(B):
        # ---------------- projections ----------------
        qpT = qp_pool.tile([CPC, SQ], BF16)
        vpT = vp_pool.tile([CPC, SK], F32)
        # zero-padded per-head K copies: K=128 matmuls run at full PE rate
        # while K=64 runs at half rate. kz0 = [kh0; 0], kz1 = [0; kh1].
        kpT_z0 = kz0_pool.tile([CPC, SK], BF16)
        kpT_z1 = kz1_pool.tile([CPC, SK], BF16)
        nc.sync.dma_start(kpT_z0[64:128, :], zeros[:, :])
        nc.sync.dma_start(kpT_z1[0:64, :], zeros[:, :])
        for jp in range(2):  # 1024-token block pairs (4KB DMA descriptors)
            jps = slice(jp * 1024, (jp + 1) * 1024)
            # v feeds both k- and v-projections
            vtiles = []
            for e in range(NE):
                t = stream.tile([128, 1024], F32R, tag="in", name=f"vin{e}")
                nc.sync.dma_start(t[:], vT[b, e * 128:(e + 1) * 128, jps])
                vtiles.append(t)
            for pname, dst in (("k", None), ("v", vpT)):
                for j2 in range(2):
                    js = slice(jp * 1024 + j2 * 512, jp * 1024 + j2 * 512 + 512)
                    rs = slice(j2 * 512, (j2 + 1) * 512)
                    pp = psum_pp.tile([128, 512], F32, tag="pp")
                    for e in range(NE):
                        nc.tensor.matmul(pp[:], w_sb[(pname, e)][:],
                                         vtiles[e][:, rs],
                                         start=(e == 0), stop=(e == NE - 1))
                    if pname == "k":
                        nc.vector.tensor_scalar_add(kpT_z0[0:64, js],
                                                    pp[0:64, :],
                                                    b_sb["k"][0:64, :])
                        nc.vector.tensor_scalar_add(kpT_z1[64:128, js],
                                                    pp[64:128, :],
                                                    b_sb["k"][64:128, :])
                    else:
                        nc.vector.tensor_scalar_add(dst[:, js], pp[:],
                                                    b_sb[pname][:])
            qtiles = []
            for e in range(NE):
                t = stream.tile([128, 1024], BF16, tag="inq", name=f"qin{e}")
                nc.sync.dma_start(t[:], qT[b, e * 128:(e + 1) * 128, jps])
                qtiles.append(t)
            for j2 in range(2):
                js = slice(jp * 1024 + j2 * 512, jp * 1024 + j2 * 512 + 512)
                rs = slice(j2 * 512, (j2 + 1) * 512)
                pp = psum_pp.tile([128, 512], F32, tag="pp")
                for e in range(NE):
                    nc.tensor.matmul(pp[:], w_sb[("q", e)][:],
                                     qtiles[e][:, rs],
                                     start=(e == 0), stop=(e == NE - 1))
                nc.vector.tensor_scalar_add(qpT[:, js], pp[:], b_sb["q"][:])

        # ---------------- v -> token-major with ones column ----------------
        # vh_aug[t]: [128 k-tok, 132]:
        #   [h0 d(0:64) | 1s | 0pad | h1 d(66:130) | 1s | 0pad]
        vh_aug = []
        for t in range(NKT):
            tp = psum_pp.tile([128, 128], F32, tag="pp")
            nc.tensor.transpose(tp[:], vpT[:, t * 128:(t + 1) * 128], id_sb[:])
            a = aug_pool.tile([128, 132], F32R, tag="aug")
            nc.vector.tensor_copy(a[:, 0:64], tp[:, 0:64])
            nc.vector.tensor_copy(a[:, 66:130], tp[:, 64:128])
            nc.vector.tensor_copy(a[:, 64:66], ones2_sb[:])
            nc.vector.tensor_copy(a[:, 130:132], ones2_sb[:])
            vh_aug.append(a)

        # ---------------- attention ----------------
        # the (PE-heavy) normalize of each qb is deferred into the next
        # qb's kt loop, where the PE has per-kt slack while ACT runs exp
        for qb in range(NQB):
            qs = slice(qb * 512, (qb + 1) * 512)
            av0 = psum_av.tile([66, 512], F32, tag="av")
            av1 = psum_av.tile([66, 512], F32, tag="av")
            for kt in range(NKT):
                ks = slice(kt * 128, (kt + 1) * 128)
                ps = psum_sc.tile([128, 1024], F32)
                # K=128 zero-padded per-head scores (full PE rate)
                nc.tensor.matmul(ps[:, 0:512], kpT_z0[:, ks],
                                 qpT[:, qs], start=True, stop=True)
                nc.tensor.matmul(ps[:, 512:1024], kpT_z1[:, ks],
                                 qpT[:, qs], start=True, stop=True)
                et = exp_pool.tile([128, 1024], F32R)
                nc.scalar.activation(et[:], ps[:], EXP, scale=SCALE)
                nc.tensor.matmul(av0[:], vh_aug[kt][:, 0:66],
                                 et[:, 0:512],
                                 start=(kt == 0), stop=(kt == NKT - 1))
                nc.tensor.matmul(av1[:], vh_aug[kt][:, 66:132],
                                 et[:, 512:1024],
                                 start=(kt == 0), stop=(kt == NKT - 1))
                if kt >= 4 and pending_norm and (kt - 4) < len(pending_norm):
                    pending_norm[kt - 4]()
                    if kt - 4 == len(pending_norm) - 1:
                        pending_norm = []
            # evict av psum now (frees banks); defer transposes/output
            avss = []
            for h, av in ((0, av0), (1, av1)):
                avs = avs_pool.tile([66, 512], F32, tag="avs")
                nc.vector.tensor_copy(avs[:], av[:])
                avss.append(avs)
            pending_norm = _make_norm(nc, tc, out_pool, psum_pp, rec_pool,
                                      out, id_sb, avss, b, qb)
        for fn in pending_norm:
            fn()
        pending_norm = []


def _make_norm(nc, tc, out_pool, psum_pp, rec_pool, out, id_sb, avss, b, qb):
    """Build deferred per-(h,t) normalize+store steps for one qb."""
    ots = [out_pool.tile([128, 128], mybir.dt.float32, tag="ot",
                         name=f"ot{t}") for t in range(4)]
    steps = []

    def make_step(h, t):
        def step():
            avs = avss[h]
            tp = psum_pp.tile([128, 66], mybir.dt.float32, tag="pp",
                              name="tpn")
            nc.tensor.transpose(tp[:], avs[:, t * 128:(t + 1) * 128],
                                id_sb[0:66, 0:66])
            rec = rec_pool.tile([128, 1], mybir.dt.float32, tag="rec",
                                name="rec")
            nc.vector.reciprocal(rec[:], tp[:, 64:65])
            nc.vector.tensor_scalar_mul(
                ots[t][:, h * 64:(h + 1) * 64], tp[:, 0:64], rec[:])
            if h == 1:
                r0 = qb * 512 + t * 128
                nc.sync.dma_start(out[b, r0:r0 + 128, :], ots[t][:])
        return step

    for t in range(4):
        for h in (0, 1):
            steps.append(make_step(h, t))
    # merge per-(h,t) into per-slot pairs so 8 slots fit in 12 kt slots
    return [(lambda a=steps[i], c=steps[i + 1]: (a(), c()))
            for i in range(0, 8, 2)]


_CACHE = {}


def _build():
    if "nc" in _CACHE:
        return _CACHE["nc"]
    nc = bacc.Bacc("TRN2", target_bir_lowering=False, debug=False,
                   enable_asserts=False)
    qT = nc.dram_tensor("qT", [B, E, SQ], BF16, kind="ExternalInput").ap()
    vT = nc.dram_tensor("vT", [B, E, SK], F32R, kind="ExternalInput").ap()
    wq = nc.dram_tensor("wq", [E, CPC], BF16, kind="ExternalInput").ap()
    wk = nc.dram_tensor("wk", [E, CPC], F32R, kind="ExternalInput").ap()
    wv = nc.dram_tensor("wv", [E, CPC], F32R, kind="ExternalInput").ap()
    bq = nc.dram_tensor("bq", [CPC, 1], F32, kind="ExternalInput").ap()
    bk = nc.dram_tensor("bk", [CPC, 1], F32, kind="ExternalInput").ap()
    bv = nc.dram_tensor("bv", [CPC, 1], F32, kind="ExternalInput").ap()
    ident = nc.dram_tensor("ident", [128, 128], F32, kind="ExternalInput").ap()
    aug_const = nc.dram_tensor("aug_const", [128, 2], F32R,
                               kind="ExternalInput").ap()
    zeros = nc.dram_tensor("zeros", [64, SK], BF16,
                           kind="ExternalInput").ap()
    out = nc.dram_tensor("out", [B, SQ, CPC], F32, kind="ExternalOutput").ap()
    with tile.TileContext(nc) as tc:
        with ExitStack() as ctx:
            _body(ctx, tc, out, qT, vT, wq, wk, wv, bq, bk, bv, ident, aug_const,
                  zeros)
    nc.compile()
    _CACHE["nc"] = nc
    return nc


def _round_tf32(x):
    """Round-to-nearest-even to 10 explicit mantissa bits (TF32)."""
    u = np.ascontiguousarray(x, np.float32).view(np.uint32)
    r = (u + 0x1000 + ((u >> 13) & 1)) & np.uint32(0xFFFFE000)
    return r.view(np.float32)


def _in_maps(q, v, Wq, bq, Wk, bk, Wv, bv):
    f = np.float32
    import ml_dtypes
    qT = np.ascontiguousarray(
        np.transpose(np.asarray(q, f), (0, 2, 1))).astype(ml_dtypes.bfloat16)
    vT = _round_tf32(np.transpose(np.asarray(v, f), (0, 2, 1)))
    ident = np.eye(128, dtype=f)
    aug2 = np.tile(np.array([1.0, 0.0], f), (128, 1))
    zeros64 = np.zeros((64, SK), ml_dtypes.bfloat16)
    Wq, Wk, Wv = (np.asarray(x, f) for x in (Wq, Wk, Wv))
    bq, bk, bv = (np.asarray(x, f) for x in (bq, bk, bv))
    maps = []
    for c in range(NCORES):
        sl = slice(c * CPC, (c + 1) * CPC)
        maps.append({
            "qT": qT, "vT": vT, "ident": ident, "aug_const": aug2,
            "zeros": zeros64,
            "wq": np.ascontiguousarray(Wq[:, sl]).astype(ml_dtypes.bfloat16),
            "wk": _round_tf32(Wk[:, sl]),
            "wv": _round_tf32(Wv[:, sl]),
            "bq": np.ascontiguousarray(bq[sl]).reshape(CPC, 1),
            "bk": np.ascontiguousarray(bk[sl]).reshape(CPC, 1),
            "bv": np.ascontiguousarray(bv[sl]).reshape(CPC, 1),
        })
    return maps


def run(trace=False, **inputs):
    nc = _build()
    maps = _in_maps(**inputs)
    res = run_bass_kernel_spmd(nc, maps, core_ids=list(range(NCORES)),
                               trace=trace)
    full = np.concatenate([res.results[c]["out"] for c in range(NCORES)],
                          axis=2)
    return full, res


def kernel(q, v, Wq, bq, Wk, bk, Wv, bv):
    full, _ = run(q=q, v=v, Wq=Wq, bq=bq, Wk=Wk, bk=bk, Wv=Wv, bv=bv)
    return full


# revision 29
# speedup vs baseline: 1.1578x; 1.0131x over previous
"""Multi-head attention (B=4, S=2048, E=1024, H=16) on 8 TRN2 NeuronCores.

Sharding: tensor-parallel over heads. Core c computes output columns
[128c, 128c+128) (heads 2c and 2c+1). Inputs q,v are fed to every core
pre-transposed to [B, E, S] so projection matmuls can use the token dim
as the moving (N=512) operand; W*/b* are column-sliced per core.

On-chip layout (per core, per batch):
  qpT/kpT/vpT [128 (= 2 heads x 64 d), 2048 tok]  -- projections, transposed
  scoresT [k_tok, q_tok] so exp+AV contraction keeps k on partitions
  softmax sums come from a ones-column appended to token-major v (M=66
  augmented AV matmul; col 65 is zero padding for the fp32r even-width
  rule); no max subtraction (scores ~ N(0, 0.25^2)).

All matmuls run in float32r (TF32-like, full PE rate at N>=512). fp32r
operands must be produced as fp32r, so matmul-feeding tiles are fp32r
and host inputs are pre-rounded.
"""

import numpy as np
from contextlib import ExitStack

import concourse.bass as bass
import concourse.tile as tile
from concourse import bacc, mybir
from concourse.bass_utils import run_bass_kernel_spmd

B, SQ, SK, E, H = 4, 2048, 2048, 1024, 16
NCORES = 8
CPC = E // NCORES          # output cols per core = 128
D = E // H                 # head dim = 64
NE = E // 128              # contraction tiles for projections = 8
NKT = SK // 128            # k-token tiles = 16
NQB = SQ // 512            # q blocks of 512 = 4
SCALE = 1.0 / np.sqrt(E)   # faithful to reference: 1/sqrt(embed_dim)

F32 = mybir.dt.float32
F32R = mybir.dt.float32r
BF16 = mybir.dt.bfloat16
EXP = mybir.ActivationFunctionType.Exp


def _body(ctx: ExitStack, tc: "tile.TileContext", out, qT, vT, wq, wk, wv,
          bq, bk, bv, ident, aug_const, zeros):
    nc = tc.nc

    const = ctx.enter_context(tc.tile_pool(name="const", bufs=1))
    stream = ctx.enter_context(tc.tile_pool(name="stream", bufs=16))
    qp_pool = ctx.enter_context(tc.tile_pool(name="qp", bufs=2))
    kp_pool = ctx.enter_context(tc.tile_pool(name="kp", bufs=2))
    kz0_pool = ctx.enter_context(tc.tile_pool(name="kz0", bufs=2))
    kz1_pool = ctx.enter_context(tc.tile_pool(name="kz1", bufs=2))
    vp_pool = ctx.enter_context(tc.tile_pool(name="vp", bufs=2))
    aug_pool = ctx.enter_context(tc.tile_pool(name="aug", bufs=20))
    exp_pool = ctx.enter_context(tc.tile_pool(name="exp", bufs=5))
    avs_pool = ctx.enter_context(tc.tile_pool(name="avs", bufs=4))
    rec_pool = ctx.enter_context(tc.tile_pool(name="rec", bufs=4))
    out_pool = ctx.enter_context(tc.tile_pool(name="outp", bufs=6))
    psum_sc = ctx.enter_context(tc.tile_pool(name="psc", bufs=2, space="PSUM"))
    psum_av = ctx.enter_context(tc.tile_pool(name="pav", bufs=2, space="PSUM"))
    psum_pp = ctx.enter_context(tc.tile_pool(name="ppp", bufs=2, space="PSUM"))

    # --- constants: weight tiles [128 (E-slice), 128 (out col)], biases, identity
    w_sb = {}
    for pname, wdram, wdt in (("q", wq, BF16), ("k", wk, F32R),
                              ("v", wv, F32R)):
        for e in range(NE):
            t = const.tile([128, CPC], wdt, tag=f"w{pname}{e}")
            nc.sync.dma_start(t[:], wdram[e * 128:(e + 1) * 128, :])
            w_sb[(pname, e)] = t
    b_sb = {}
    for pname, bdram in (("q", bq), ("k", bk), ("v", bv)):
        t = const.tile([CPC, 1], F32, tag=f"b{pname}")
        nc.sync.dma_start(t[:], bdram[:, :])
        b_sb[pname] = t
    id_sb = const.tile([128, 128], F32, tag="ident")
    nc.sync.dma_start(id_sb[:], ident[:, :])
    # [1, 0] per partition, fp32r (memset cannot produce fp32r)
    ones2_sb = const.tile([128, 2], F32R, tag="ones2")
    nc.sync.dma_start(ones2_sb[:], aug_const[:, :])

    pending_norm = []
    for b in range(B):
        # ---------------- projections ----------------
        qpT = qp_pool.tile([CPC, SQ], BF16)
        vpT = vp_pool.tile([CPC, SK], F32)
        # zero-padded per-head K copies: K=128 matmuls run at full PE rate
        # while K=64 runs at half rate. kz0 = [kh0; 0], kz1 = [0; kh1].
        kpT_z0 = kz0_pool.tile([CPC, SK], BF16)
        kpT_z1 = kz1_pool.tile([CPC, SK], BF16)
        for jp in range(2):  # 1024-token block pairs (4KB DMA descriptors)
            jps = slice(jp * 1024, (jp + 1) * 1024)
            # v feeds both k- and v-projections
            vtiles = []
            for e in range(NE):
                t = stream.tile([128, 1024], F32R, tag="in", name=f"vin{e}")
                nc.sync.dma_start(t[:], vT[b, e * 128:(e + 1) * 128, jps])
                vtiles.append(t)
            for pname, dst in (("k", None), ("v", vpT)):
                for j2 in range(2):
                    js = slice(jp * 1024 + j2 * 512, jp * 1024 + j2 * 512 + 512)
                    rs = slice(j2 * 512, (j2 + 1) * 512)
                    pp = psum_pp.tile([128, 512], F32, tag="pp")
                    for e in range(NE):
                        nc.tensor.matmul(pp[:], w_sb[(pname, e)][:],
                                         vtiles[e][:, rs],
                                         start=(e == 0), stop=(e == NE - 1))
                    if pname == "k":
                        nc.vector.tensor_scalar_add(kpT_z0[0:64, js],
                                                    pp[0:64, :],
                                                    b_sb["k"][0:64, :])
                        nc.vector.tensor_scalar_add(kpT_z1[64:128, js],
                                                    pp[64:128, :],
                                                    b_sb["k"][64:128, :])
                    else:
                        nc.vector.tensor_scalar_add(dst[:, js], pp[:],
                                                    b_sb[pname][:])
            qtiles = []
            for e in range(NE):
                t = stream.tile([128, 1024], BF16, tag="inq", name=f"qin{e}")
                nc.sync.dma_start(t[:], qT[b, e * 128:(e + 1) * 128, jps])
                qtiles.append(t)
            for j2 in range(2):
                js = slice(jp * 1024 + j2 * 512, jp * 1024 + j2 * 512 + 512)
                rs = slice(j2 * 512, (j2 + 1) * 512)
                pp = psum_pp.tile([128, 512], F32, tag="pp")
                for e in range(NE):
                    nc.tensor.matmul(pp[:], w_sb[("q", e)][:],
                                     qtiles[e][:, rs],
                                     start=(e == 0), stop=(e == NE - 1))
                nc.vector.tensor_scalar_add(qpT[:, js], pp[:], b_sb["q"][:])

        nc.sync.dma_start(kpT_z0[64:128, :], zeros[:, :])
        nc.sync.dma_start(kpT_z1[0:64, :], zeros[:, :])

        # ---------------- v -> token-major with ones column ----------------
        # vh_aug[t]: [128 k-tok, 132]:
        #   [h0 d(0:64) | 1s | 0pad | h1 d(66:130) | 1s | 0pad]
        vh_aug = []
        for t in range(NKT):
            tp = psum_pp.tile([128, 128], F32, tag="pp")
            nc.tensor.transpose(tp[:], vpT[:, t * 128:(t + 1) * 128], id_sb[:])
            a = aug_pool.tile([128, 132], F32R, tag="aug")
            nc.vector.tensor_copy(a[:, 0:64], tp[:, 0:64])
            nc.vector.tensor_copy(a[:, 66:130], tp[:, 64:128])
            nc.vector.tensor_copy(a[:, 64:66], ones2_sb[:])
            nc.vector.tensor_copy(a[:, 130:132], ones2_sb[:])
            vh_aug.append(a)

        # ---------------- attention ----------------
        # the (PE-heavy) normalize of each qb is deferred into the next
        # qb's kt loop, where the PE has per-kt slack while ACT runs exp
        for qb in range(NQB):
            qs = slice(qb * 512, (qb + 1) * 512)
            av0 = psum_av.tile([66, 512], F32, tag="av")
            av1 = psum_av.tile([66, 512], F32, tag="av")
            for kt in range(NKT):
                ks = slice(kt * 128, (kt + 1) * 128)
                ps = psum_sc.tile([128, 1024], F32)
                # K=128 zero-padded per-head scores (full PE rate)
                nc.tensor.matmul(ps[:, 0:512], kpT_z0[:, ks],
                                 qpT[:, qs], start=True, stop=True)
                nc.tensor.matmul(ps[:, 512:1024], kpT_z1[:, ks],
                                 qpT[:, qs], start=True, stop=True)
                et = exp_pool.tile([128, 1024], F32R)
                nc.scalar.activation(et[:], ps[:], EXP, scale=SCALE)
                nc.tensor.matmul(av0[:], vh_aug[kt][:, 0:66],
                                 et[:, 0:512],
                                 start=(kt == 0), stop=(kt == NKT - 1))
                nc.tensor.matmul(av1[:], vh_aug[kt][:, 66:132],
                                 et[:, 512:1024],
                                 start=(kt == 0), stop=(kt == NKT - 1))
                if kt >= 4 and pending_norm and (kt - 4) < len(pending_norm):
                    pending_norm[kt - 4]()
                    if kt - 4 == len(pending_norm) - 1:
                        pending_norm = []
            # evict av psum now (frees banks); defer transposes/output
            avss = []
            for h, av in ((0, av0), (1, av1)):
                avs = avs_pool.tile([66, 512], F32, tag="avs")
                nc.vector.tensor_copy(avs[:], av[:])
                avss.append(avs)
            pending_norm = _make_norm(nc, tc, out_pool, psum_pp, rec_pool,
                                      out, id_sb, avss, b, qb)
        for fn in pending_norm:
            fn()
        pending_norm = []


def _make_norm(nc, tc, out_pool, psum_pp, rec_pool, out, id_sb, avss, b, qb):
    """Build deferred per-(h,t) normalize+store steps for one qb."""
    ots = [out_pool.tile([128, 128], mybir.dt.float32, tag="ot",
                         name=f"ot{t}") for t in range(4)]
    steps = []

    def make_step(h, t):
        def step():
            avs = avss[h]
            tp = psum_pp.tile([128, 66], mybir.dt.float32, tag="pp",
                              name="tpn")
            nc.tensor.transpose(tp[:], avs[:, t * 128:(t + 1) * 128],
                                id_sb[0:66, 0:66])
            rec = rec_pool.tile([128, 1], mybir.dt.float32, tag="rec",
                                name="rec")
            nc.vector.reciprocal(rec[:], tp[:, 64:65])
            nc.vector.tensor_scalar_mul(
                ots[t][:, h * 64:(h + 1) * 64], tp[:, 0:64], rec[:])
            if h == 1:
                r0 = qb * 512 + t * 128
                nc.sync.dma_start(out[b, r0:r0 + 128, :], ots[t][:])
        return step

    for t in range(4):
        for h in (0, 1):
            steps.append(make_step(h, t))
    # merge per-(h,t) into per-slot pairs so 8 slots fit in 12 kt slots
    return [(lambda a=steps[i], c=steps[i + 1]: (a(), c()))
            for i in range(0, 8, 2)]


_CACHE = {}


def _build():
    if "nc" in _CACHE:
        return _CACHE["nc"]
    nc = bacc.Bacc("TRN2", target_bir_lowering=False, debug=False,
                   enable_asserts=False)
    qT = nc.dram_tensor("qT", [B, E, SQ], BF16, kind="ExternalInput").ap()
    vT = nc.dram_tensor("vT", [B, E, SK], F32R, kind="ExternalInput").ap()
    wq = nc.dram_tensor("wq", [E, CPC], BF16, kind="ExternalInput").ap()
    wk = nc.dram_tensor("wk", [E, CPC], F32R, kind="ExternalInput").ap()
    wv = nc.dram_tensor("wv", [E, CPC], F32R, kind="ExternalInput").ap()
    bq = nc.dram_tensor("bq", [CPC, 1], F32, kind="ExternalInput").ap()
    bk = nc.dram_tensor("bk", [CPC, 1], F32, kind="ExternalInput").ap()
    bv = nc.dram_tensor("bv", [CPC, 1], F32, kind="ExternalInput").ap()
    ident = nc.dram_tensor("ident", [128, 128], F32, kind="ExternalInput").ap()
    aug_const = nc.dram_tensor("aug_const", [128, 2], F32R,
                               kind="ExternalInput").ap()
    zeros = nc.dram_tensor("zeros", [64, SK], BF16,
                           kind="ExternalInput").ap()
    out = nc.dram_tensor("out", [B, SQ, CPC], F32, kind="ExternalOutput").ap()
    with tile.TileContext(nc) as tc:
        with ExitStack() as ctx:
            _body(ctx, tc, out, qT, vT, wq, wk, wv, bq, bk, bv, ident, aug_const,
                  zeros)
    nc.compile()
    _CACHE["nc"] = nc
    return nc


def _round_tf32(x):
    """Round-to-nearest-even to 10 explicit mantissa bits (TF32)."""
    u = np.ascontiguousarray(x, np.float32).view(np.uint32)
    r = (u + 0x1000 + ((u >> 13) & 1)) & np.uint32(0xFFFFE000)
    return r.view(np.float32)


def _in_maps(q, v, Wq, bq, Wk, bk, Wv, bv):
    f = np.float32
    import ml_dtypes
    qT = np.ascontiguousarray(
        np.transpose(np.asarray(q, f), (0, 2, 1))).astype(ml_dtypes.bfloat16)
    vT = _round_tf32(np.transpose(np.asarray(v, f), (0, 2, 1)))
    ident = np.eye(128, dtype=f)
    aug2 = np.tile(np.array([1.0, 0.0], f), (128, 1))
    zeros64 = np.zeros((64, SK), ml_dtypes.bfloat16)
    Wq, Wk, Wv = (np.asarray(x, f) for x in (Wq, Wk, Wv))
    bq, bk, bv = (np.asarray(x, f) for x in (bq, bk, bv))
    maps = []
    for c in range(NCORES):
        sl = slice(c * CPC, (c + 1) * CPC)
        maps.append({
            "qT": qT, "vT": vT, "ident": ident, "aug_const": aug2,
            "zeros": zeros64,
            "wq": np.ascontiguousarray(Wq[:, sl]).astype(ml_dtypes.bfloat16),
            "wk": _round_tf32(Wk[:, sl]),
            "wv": _round_tf32(Wv[:, sl]),
            "bq": np.ascontiguousarray(bq[sl]).reshape(CPC, 1),
            "bk": np.ascontiguousarray(bk[sl]).reshape(CPC, 1),
            "bv": np.ascontiguousarray(bv[sl]).reshape(CPC, 1),
        })
    return maps


def run(trace=False, **inputs):
    nc = _build()
    maps = _in_maps(**inputs)
    res = run_bass_kernel_spmd(nc, maps, core_ids=list(range(NCORES)),
                               trace=trace)
    full = np.concatenate([res.results[c]["out"] for c in range(NCORES)],
                          axis=2)
    return full, res


def kernel(q, v, Wq, bq, Wk, bk, Wv, bv):
    full, _ = run(q=q, v=v, Wq=Wq, bq=bq, Wk=Wk, bk=bk, Wv=Wv, bv=bv)
    return full


# revision 30
# speedup vs baseline: 1.1678x; 1.0087x over previous
"""Multi-head attention (B=4, S=2048, E=1024, H=16) on 8 TRN2 NeuronCores.

Sharding: tensor-parallel over heads. Core c computes output columns
[128c, 128c+128) (heads 2c and 2c+1). Inputs q,v are fed to every core
pre-transposed to [B, E, S] so projection matmuls can use the token dim
as the moving (N=512) operand; W*/b* are column-sliced per core.

On-chip layout (per core, per batch):
  qpT/kpT/vpT [128 (= 2 heads x 64 d), 2048 tok]  -- projections, transposed
  scoresT [k_tok, q_tok] so exp+AV contraction keeps k on partitions
  softmax sums come from a ones-column appended to token-major v (M=66
  augmented AV matmul; col 65 is zero padding for the fp32r even-width
  rule); no max subtraction (scores ~ N(0, 0.25^2)).

All matmuls run in float32r (TF32-like, full PE rate at N>=512). fp32r
operands must be produced as fp32r, so matmul-feeding tiles are fp32r
and host inputs are pre-rounded.
"""

import numpy as np
from contextlib import ExitStack

import concourse.bass as bass
import concourse.tile as tile
from concourse import bacc, mybir
from concourse.bass_utils import run_bass_kernel_spmd

B, SQ, SK, E, H = 4, 2048, 2048, 1024, 16
NCORES = 8
CPC = E // NCORES          # output cols per core = 128
D = E // H                 # head dim = 64
NE = E // 128              # contraction tiles for projections = 8
NKT = SK // 128            # k-token tiles = 16
NQB = SQ // 512            # q blocks of 512 = 4
SCALE = 1.0 / np.sqrt(E)   # faithful to reference: 1/sqrt(embed_dim)

F32 = mybir.dt.float32
F32R = mybir.dt.float32r
BF16 = mybir.dt.bfloat16
EXP = mybir.ActivationFunctionType.Exp


def _body(ctx: ExitStack, tc: "tile.TileContext", out, qT, vT, wq, wk, wv,
          bq, bk, bv, ident, aug_const, zeros):
    nc = tc.nc

    const = ctx.enter_context(tc.tile_pool(name="const", bufs=1))
    stream = ctx.enter_context(tc.tile_pool(name="stream", bufs=16))
    qp_pool = ctx.enter_context(tc.tile_pool(name="qp", bufs=2))
    kp_pool = ctx.enter_context(tc.tile_pool(name="kp", bufs=2))
    kz0_pool = ctx.enter_context(tc.tile_pool(name="kz0", bufs=2))
    kz1_pool = ctx.enter_context(tc.tile_pool(name="kz1", bufs=2))
    vp_pool = ctx.enter_context(tc.tile_pool(name="vp", bufs=2))
    aug_pool = ctx.enter_context(tc.tile_pool(name="aug", bufs=20))
    exp_pool = ctx.enter_context(tc.tile_pool(name="exp", bufs=5))
    avs_pool = ctx.enter_context(tc.tile_pool(name="avs", bufs=4))
    rec_pool = ctx.enter_context(tc.tile_pool(name="rec", bufs=4))
    out_pool = ctx.enter_context(tc.tile_pool(name="outp", bufs=6))
    psum_sc = ctx.enter_context(tc.tile_pool(name="psc", bufs=2, space="PSUM"))
    psum_av = ctx.enter_context(tc.tile_pool(name="pav", bufs=2, space="PSUM"))
    psum_pp = ctx.enter_context(tc.tile_pool(name="ppp", bufs=2, space="PSUM"))

    # --- constants: weight tiles [128 (E-slice), 128 (out col)], biases, identity
    w_sb = {}
    # k weights first: the first projection chains need them immediately
    for pname, wdram, wdt in (("k", wk, F32R), ("v", wv, F32R),
                              ("q", wq, BF16)):
        for e in range(NE):
            t = const.tile([128, CPC], wdt, tag=f"w{pname}{e}")
            nc.sync.dma_start(t[:], wdram[e * 128:(e + 1) * 128, :])
            w_sb[(pname, e)] = t
    b_sb = {}
    for pname, bdram in (("q", bq), ("k", bk), ("v", bv)):
        t = const.tile([CPC, 1], F32, tag=f"b{pname}")
        nc.sync.dma_start(t[:], bdram[:, :])
        b_sb[pname] = t
    id_sb = const.tile([128, 128], F32, tag="ident")
    nc.sync.dma_start(id_sb[:], ident[:, :])
    # [1, 0] per partition, fp32r (memset cannot produce fp32r)
    ones2_sb = const.tile([128, 2], F32R, tag="ones2")
    nc.sync.dma_start(ones2_sb[:], aug_const[:, :])

    pending_norm = []
    for b in range(B):
        # ---------------- projections ----------------
        qpT = qp_pool.tile([CPC, SQ], BF16)
        vpT = vp_pool.tile([CPC, SK], F32)
        # zero-padded per-head K copies: K=128 matmuls run at full PE rate
        # while K=64 runs at half rate. kz0 = [kh0; 0], kz1 = [0; kh1].
        kpT_z0 = kz0_pool.tile([CPC, SK], BF16)
        kpT_z1 = kz1_pool.tile([CPC, SK], BF16)
        for jp in range(2):  # 1024-token block pairs (4KB DMA descriptors)
            jps = slice(jp * 1024, (jp + 1) * 1024)
            # v feeds both k- and v-projections
            vtiles = []
            for e in range(NE):
                t = stream.tile([128, 1024], F32R, tag="in", name=f"vin{e}")
                nc.sync.dma_start(t[:], vT[b, e * 128:(e + 1) * 128, jps])
                vtiles.append(t)
            for pname, dst in (("k", None), ("v", vpT)):
                for j2 in range(2):
                    js = slice(jp * 1024 + j2 * 512, jp * 1024 + j2 * 512 + 512)
                    rs = slice(j2 * 512, (j2 + 1) * 512)
                    pp = psum_pp.tile([128, 512], F32, tag="pp")
                    for e in range(NE):
                        nc.tensor.matmul(pp[:], w_sb[(pname, e)][:],
                                         vtiles[e][:, rs],
                                         start=(e == 0), stop=(e == NE - 1))
                    if pname == "k":
                        nc.vector.tensor_scalar_add(kpT_z0[0:64, js],
                                                    pp[0:64, :],
                                                    b_sb["k"][0:64, :])
                        nc.vector.tensor_scalar_add(kpT_z1[64:128, js],
                                                    pp[64:128, :],
                                                    b_sb["k"][64:128, :])
                    else:
                        nc.vector.tensor_scalar_add(dst[:, js], pp[:],
                                                    b_sb[pname][:])
            qtiles = []
            for e in range(NE):
                t = stream.tile([128, 1024], BF16, tag="inq", name=f"qin{e}")
                nc.sync.dma_start(t[:], qT[b, e * 128:(e + 1) * 128, jps])
                qtiles.append(t)
            for j2 in range(2):
                js = slice(jp * 1024 + j2 * 512, jp * 1024 + j2 * 512 + 512)
                rs = slice(j2 * 512, (j2 + 1) * 512)
                pp = psum_pp.tile([128, 512], F32, tag="pp")
                for e in range(NE):
                    nc.tensor.matmul(pp[:], w_sb[("q", e)][:],
                                     qtiles[e][:, rs],
                                     start=(e == 0), stop=(e == NE - 1))
                nc.vector.tensor_scalar_add(qpT[:, js], pp[:], b_sb["q"][:])

        nc.sync.dma_start(kpT_z0[64:128, :], zeros[:, :])
        nc.sync.dma_start(kpT_z1[0:64, :], zeros[:, :])

        # ---------------- v -> token-major with ones column ----------------
        # vh_aug[t]: [128 k-tok, 132]:
        #   [h0 d(0:64) | 1s | 0pad | h1 d(66:130) | 1s | 0pad]
        vh_aug = []
        for t in range(NKT):
            tp = psum_pp.tile([128, 128], F32, tag="pp")
            nc.tensor.transpose(tp[:], vpT[:, t * 128:(t + 1) * 128], id_sb[:])
            a = aug_pool.tile([128, 132], F32R, tag="aug")
            nc.vector.tensor_copy(a[:, 0:64], tp[:, 0:64])
            nc.vector.tensor_copy(a[:, 66:130], tp[:, 64:128])
            nc.vector.tensor_copy(a[:, 64:66], ones2_sb[:])
            nc.vector.tensor_copy(a[:, 130:132], ones2_sb[:])
            vh_aug.append(a)

        # ---------------- attention ----------------
        # the (PE-heavy) normalize of each qb is deferred into the next
        # qb's kt loop, where the PE has per-kt slack while ACT runs exp
        for qb in range(NQB):
            qs = slice(qb * 512, (qb + 1) * 512)
            av0 = psum_av.tile([66, 512], F32, tag="av")
            av1 = psum_av.tile([66, 512], F32, tag="av")
            for kt in range(NKT):
                ks = slice(kt * 128, (kt + 1) * 128)
                ps = psum_sc.tile([128, 1024], F32)
                # K=128 zero-padded per-head scores (full PE rate)
                nc.tensor.matmul(ps[:, 0:512], kpT_z0[:, ks],
                                 qpT[:, qs], start=True, stop=True)
                nc.tensor.matmul(ps[:, 512:1024], kpT_z1[:, ks],
                                 qpT[:, qs], start=True, stop=True)
                et = exp_pool.tile([128, 1024], F32R)
                nc.scalar.activation(et[:], ps[:], EXP, scale=SCALE)
                nc.tensor.matmul(av0[:], vh_aug[kt][:, 0:66],
                                 et[:, 0:512],
                                 start=(kt == 0), stop=(kt == NKT - 1))
                nc.tensor.matmul(av1[:], vh_aug[kt][:, 66:132],
                                 et[:, 512:1024],
                                 start=(kt == 0), stop=(kt == NKT - 1))
                if kt >= 4 and pending_norm and (kt - 4) < len(pending_norm):
                    pending_norm[kt - 4]()
                    if kt - 4 == len(pending_norm) - 1:
                        pending_norm = []
            # evict av psum now (frees banks); defer transposes/output
            avss = []
            for h, av in ((0, av0), (1, av1)):
                avs = avs_pool.tile([66, 512], F32, tag="avs")
                nc.vector.tensor_copy(avs[:], av[:])
                avss.append(avs)
            pending_norm = _make_norm(nc, tc, out_pool, psum_pp, rec_pool,
                                      out, id_sb, avss, b, qb)
        for fn in pending_norm:
            fn()
        pending_norm = []


def _make_norm(nc, tc, out_pool, psum_pp, rec_pool, out, id_sb, avss, b, qb):
    """Build deferred per-(h,t) normalize+store steps for one qb."""
    ots = [out_pool.tile([128, 128], mybir.dt.float32, tag="ot",
                         name=f"ot{t}") for t in range(4)]
    steps = []

    def make_step(h, t):
        def step():
            avs = avss[h]
            tp = psum_pp.tile([128, 66], mybir.dt.float32, tag="pp",
                              name="tpn")
            nc.tensor.transpose(tp[:], avs[:, t * 128:(t + 1) * 128],
                                id_sb[0:66, 0:66])
            rec = rec_pool.tile([128, 1], mybir.dt.float32, tag="rec",
                                name="rec")
            nc.vector.reciprocal(rec[:], tp[:, 64:65])
            nc.vector.tensor_scalar_mul(
                ots[t][:, h * 64:(h + 1) * 64], tp[:, 0:64], rec[:])
            if h == 1:
                r0 = qb * 512 + t * 128
                nc.sync.dma_start(out[b, r0:r0 + 128, :], ots[t][:])
        return step

    for t in range(4):
        for h in (0, 1):
            steps.append(make_step(h, t))
    # merge per-(h,t) into per-slot pairs so 8 slots fit in 12 kt slots
    return [(lambda a=steps[i], c=steps[i + 1]: (a(), c()))
            for i in range(0, 8, 2)]


_CACHE = {}


def _build():
    if "nc" in _CACHE:
        return _CACHE["nc"]
    nc = bacc.Bacc("TRN2", target_bir_lowering=False, debug=False,
                   enable_asserts=False)
    qT = nc.dram_tensor("qT", [B, E, SQ], BF16, kind="ExternalInput").ap()
    vT = nc.dram_tensor("vT", [B, E, SK], F32R, kind="ExternalInput").ap()
    wq = nc.dram_tensor("wq", [E, CPC], BF16, kind="ExternalInput").ap()
    wk = nc.dram_tensor("wk", [E, CPC], F32R, kind="ExternalInput").ap()
    wv = nc.dram_tensor("wv", [E, CPC], F32R, kind="ExternalInput").ap()
    bq = nc.dram_tensor("bq", [CPC, 1], F32, kind="ExternalInput").ap()
    bk = nc.dram_tensor("bk", [CPC, 1], F32, kind="ExternalInput").ap()
    bv = nc.dram_tensor("bv", [CPC, 1], F32, kind="ExternalInput").ap()
    ident = nc.dram_tensor("ident", [128, 128], F32, kind="ExternalInput").ap()
    aug_const = nc.dram_tensor("aug_const", [128, 2], F32R,
                               kind="ExternalInput").ap()
    zeros = nc.dram_tensor("zeros", [64, SK], BF16,
                           kind="ExternalInput").ap()
    out = nc.dram_tensor("out", [B, SQ, CPC], F32, kind="ExternalOutput").ap()
    with tile.TileContext(nc) as tc:
        with ExitStack() as ctx:
            _body(ctx, tc, out, qT, vT, wq, wk, wv, bq, bk, bv, ident, aug_const,
                  zeros)
    nc.compile()
    _CACHE["nc"] = nc
    return nc


def _round_tf32(x):
    """Round-to-nearest-even to 10 explicit mantissa bits (TF32)."""
    u = np.ascontiguousarray(x, np.float32).view(np.uint32)
    r = (u + 0x1000 + ((u >> 13) & 1)) & np.uint32(0xFFFFE000)
    return r.view(np.float32)


def _in_maps(q, v, Wq, bq, Wk, bk, Wv, bv):
    f = np.float32
    import ml_dtypes
    qT = np.ascontiguousarray(
        np.transpose(np.asarray(q, f), (0, 2, 1))).astype(ml_dtypes.bfloat16)
    vT = _round_tf32(np.transpose(np.asarray(v, f), (0, 2, 1)))
    ident = np.eye(128, dtype=f)
    aug2 = np.tile(np.array([1.0, 0.0], f), (128, 1))
    zeros64 = np.zeros((64, SK), ml_dtypes.bfloat16)
    Wq, Wk, Wv = (np.asarray(x, f) for x in (Wq, Wk, Wv))
    bq, bk, bv = (np.asarray(x, f) for x in (bq, bk, bv))
    maps = []
    for c in range(NCORES):
        sl = slice(c * CPC, (c + 1) * CPC)
        maps.append({
            "qT": qT, "vT": vT, "ident": ident, "aug_const": aug2,
            "zeros": zeros64,
            "wq": np.ascontiguousarray(Wq[:, sl]).astype(ml_dtypes.bfloat16),
            "wk": _round_tf32(Wk[:, sl]),
            "wv": _round_tf32(Wv[:, sl]),
            "bq": np.ascontiguousarray(bq[sl]).reshape(CPC, 1),
            "bk": np.ascontiguousarray(bk[sl]).reshape(CPC, 1),
            "bv": np.ascontiguousarray(bv[sl]).reshape(CPC, 1),
        })
    return maps


def run(trace=False, **inputs):
    nc = _build()
    maps = _in_maps(**inputs)
    res = run_bass_kernel_spmd(nc, maps, core_ids=list(range(NCORES)),
                               trace=trace)
    full = np.concatenate([res.results[c]["out"] for c in range(NCORES)],
                          axis=2)
    return full, res


def kernel(q, v, Wq, bq, Wk, bk, Wv, bv):
    full, _ = run(q=q, v=v, Wq=Wq, bq=bq, Wk=Wk, bk=bk, Wv=Wv, bv=bv)
    return full


# revision 31
# speedup vs baseline: 1.2205x; 1.0451x over previous
"""Multi-head attention (B=4, S=2048, E=1024, H=16) on 8 TRN2 NeuronCores.

Sharding: tensor-parallel over heads. Core c computes output columns
[128c, 128c+128) (heads 2c and 2c+1). Inputs q,v are fed to every core
pre-transposed to [B, E, S] so projection matmuls can use the token dim
as the moving (N=512) operand; W*/b* are column-sliced per core.

On-chip layout (per core, per batch):
  qpT/kpT/vpT [128 (= 2 heads x 64 d), 2048 tok]  -- projections, transposed
  scoresT [k_tok, q_tok] so exp+AV contraction keeps k on partitions
  softmax sums come from a ones-column appended to token-major v (M=66
  augmented AV matmul; col 65 is zero padding for the fp32r even-width
  rule); no max subtraction (scores ~ N(0, 0.25^2)).

All matmuls run in float32r (TF32-like, full PE rate at N>=512). fp32r
operands must be produced as fp32r, so matmul-feeding tiles are fp32r
and host inputs are pre-rounded.
"""

import numpy as np
from contextlib import ExitStack

import concourse.bass as bass
import concourse.tile as tile
from concourse import bacc, mybir
from concourse.bass_utils import run_bass_kernel_spmd

B, SQ, SK, E, H = 4, 2048, 2048, 1024, 16
NCORES = 8
CPC = E // NCORES          # output cols per core = 128
D = E // H                 # head dim = 64
NE = E // 128              # contraction tiles for projections = 8
NKT = SK // 128            # k-token tiles = 16
NQB = SQ // 512            # q blocks of 512 = 4
SCALE = 1.0 / np.sqrt(E)   # faithful to reference: 1/sqrt(embed_dim)

F32 = mybir.dt.float32
F32R = mybir.dt.float32r
BF16 = mybir.dt.bfloat16
EXP = mybir.ActivationFunctionType.Exp


def _body(ctx: ExitStack, tc: "tile.TileContext", out, qT, vT, wq, wk, wv,
          bq, bk, bv, ident, aug_const, zeros):
    nc = tc.nc

    const = ctx.enter_context(tc.tile_pool(name="const", bufs=1))
    stream = ctx.enter_context(tc.tile_pool(name="stream", bufs=13))
    qp_pool = ctx.enter_context(tc.tile_pool(name="qp", bufs=2))
    kp_pool = ctx.enter_context(tc.tile_pool(name="kp", bufs=2))
    kz0_pool = ctx.enter_context(tc.tile_pool(name="kz0", bufs=2))
    kz1_pool = ctx.enter_context(tc.tile_pool(name="kz1", bufs=2))
    vp_pool = ctx.enter_context(tc.tile_pool(name="vp", bufs=2))
    aug_pool = ctx.enter_context(tc.tile_pool(name="aug", bufs=32))
    exp_pool = ctx.enter_context(tc.tile_pool(name="exp", bufs=5))
    avs_pool = ctx.enter_context(tc.tile_pool(name="avs", bufs=4))
    rec_pool = ctx.enter_context(tc.tile_pool(name="rec", bufs=4))
    out_pool = ctx.enter_context(tc.tile_pool(name="outp", bufs=6))
    psum_sc = ctx.enter_context(tc.tile_pool(name="psc", bufs=2, space="PSUM"))
    psum_av = ctx.enter_context(tc.tile_pool(name="pav", bufs=2, space="PSUM"))
    psum_pp = ctx.enter_context(tc.tile_pool(name="ppp", bufs=2, space="PSUM"))

    # --- constants: weight tiles [128 (E-slice), 128 (out col)], biases, identity
    w_sb = {}
    # k weights first: the first projection chains need them immediately
    for pname, wdram, wdt in (("k", wk, F32R), ("v", wv, F32R),
                              ("q", wq, BF16)):
        for e in range(NE):
            t = const.tile([128, CPC], wdt, tag=f"w{pname}{e}")
            nc.sync.dma_start(t[:], wdram[e * 128:(e + 1) * 128, :])
            w_sb[(pname, e)] = t
    b_sb = {}
    for pname, bdram in (("q", bq), ("k", bk), ("v", bv)):
        t = const.tile([CPC, 1], F32, tag=f"b{pname}")
        nc.sync.dma_start(t[:], bdram[:, :])
        b_sb[pname] = t
    id_sb = const.tile([128, 128], F32, tag="ident")
    nc.sync.dma_start(id_sb[:], ident[:, :])
    # [1, 0] per partition, fp32r (memset cannot produce fp32r)
    ones2_sb = const.tile([128, 2], F32R, tag="ones2")
    nc.sync.dma_start(ones2_sb[:], aug_const[:, :])

    pending_norm = []

    def emit_batch_proj(b):
        """Emit-later chunks for batch b's projections + vh_aug build.

        Returns (chunks, handles) where handles fills in as chunks run.
        Chunks are emitted interleaved into the previous batch's attention
        so the ACT engine never starves while the in-order PE does proj.
        """
        h = {}
        chunks = []

        def alloc():
            h["qpT"] = qp_pool.tile([CPC, SQ], BF16, name="qpT")
            h["vpT"] = vp_pool.tile([CPC, SK], F32, name="vpT")
            h["kz0"] = kz0_pool.tile([CPC, SK], BF16, name="kz0")
            h["kz1"] = kz1_pool.tile([CPC, SK], BF16, name="kz1")
            h["aug"] = []
            nc.sync.dma_start(h["kz0"][64:128, :], zeros[:, :])
            nc.sync.dma_start(h["kz1"][0:64, :], zeros[:, :])
        chunks.append(alloc)

        def load_v(jp):
            def go():
                jps = slice(jp * 1024, (jp + 1) * 1024)
                h[f"v{jp}"] = []
                for e in range(NE):
                    t = stream.tile([128, 1024], F32R, tag="in",
                                    name=f"vin{e}")
                    nc.sync.dma_start(t[:], vT[b, e * 128:(e + 1) * 128, jps])
                    h[f"v{jp}"].append(t)
            return go

        def load_q(jp):
            def go():
                jps = slice(jp * 1024, (jp + 1) * 1024)
                h[f"q{jp}"] = []
                for e in range(NE):
                    t = stream.tile([128, 1024], BF16, tag="inq",
                                    name=f"qin{e}")
                    nc.sync.dma_start(t[:], qT[b, e * 128:(e + 1) * 128, jps])
                    h[f"q{jp}"].append(t)
            return go

        def chain(pname, jp, j2):
            def go():
                js = slice(jp * 1024 + j2 * 512, jp * 1024 + j2 * 512 + 512)
                rs = slice(j2 * 512, (j2 + 1) * 512)
                tiles = h[f"q{jp}"] if pname == "q" else h[f"v{jp}"]
                pp = psum_pp.tile([128, 512], F32, tag="pp", name="pp")
                for e in range(NE):
                    nc.tensor.matmul(pp[:], w_sb[(pname, e)][:],
                                     tiles[e][:, rs],
                                     start=(e == 0), stop=(e == NE - 1))
                if pname == "k":
                    nc.vector.tensor_scalar_add(h["kz0"][0:64, js],
                                                pp[0:64, :],
                                                b_sb["k"][0:64, :])
                    nc.vector.tensor_scalar_add(h["kz1"][64:128, js],
                                                pp[64:128, :],
                                                b_sb["k"][64:128, :])
                elif pname == "q":
                    nc.vector.tensor_scalar_add(h["qpT"][:, js], pp[:],
                                                b_sb["q"][:])
                else:
                    nc.vector.tensor_scalar_add(h["vpT"][:, js], pp[:],
                                                b_sb["v"][:])
            return go

        def augt(t):
            def go():
                tp = psum_pp.tile([128, 128], F32, tag="pp", name="tpv")
                nc.tensor.transpose(tp[:], h["vpT"][:, t * 128:(t + 1) * 128],
                                    id_sb[:])
                a = aug_pool.tile([128, 132], F32R, tag="aug", name="aug")
                nc.vector.tensor_copy(a[:, 0:64], tp[:, 0:64])
                nc.vector.tensor_copy(a[:, 66:130], tp[:, 64:128])
                nc.vector.tensor_copy(a[:, 64:66], ones2_sb[:])
                nc.vector.tensor_copy(a[:, 130:132], ones2_sb[:])
                h["aug"].append(a)
            return go

        for jp in range(2):
            chunks.append(load_v(jp))
            chunks.append(load_q(jp))
            for j2 in range(2):
                chunks.append(chain("k", jp, j2))
                chunks.append(chain("q", jp, j2))
            for j2 in range(2):
                chunks.append(chain("v", jp, j2))
        for t in range(0, NKT, 2):
            chunks.append(lambda t=t: (augt(t)(), augt(t + 1)()))
        return chunks, h

    # prologue: batch 0 projections emitted immediately
    chunks, cur = emit_batch_proj(0)
    for c in chunks:
        c()

    for b in range(B):
        qpT = cur["qpT"]
        kpT_z0 = cur["kz0"]
        kpT_z1 = cur["kz1"]
        vh_aug = cur["aug"]
        next_chunks, nxt = (emit_batch_proj(b + 1) if b + 1 < B
                            else ([], None))
        ci = 0

        # ---------------- attention ----------------
        for qb in range(NQB):
            qs = slice(qb * 512, (qb + 1) * 512)
            av0 = psum_av.tile([66, 512], F32, tag="av")
            av1 = psum_av.tile([66, 512], F32, tag="av")
            for kt in range(NKT):
                ks = slice(kt * 128, (kt + 1) * 128)
                ps = psum_sc.tile([128, 1024], F32)
                # K=128 zero-padded per-head scores (full PE rate)
                nc.tensor.matmul(ps[:, 0:512], kpT_z0[:, ks],
                                 qpT[:, qs], start=True, stop=True)
                nc.tensor.matmul(ps[:, 512:1024], kpT_z1[:, ks],
                                 qpT[:, qs], start=True, stop=True)
                et = exp_pool.tile([128, 1024], F32R)
                nc.scalar.activation(et[:], ps[:], EXP, scale=SCALE)
                nc.tensor.matmul(av0[:], vh_aug[kt][:, 0:66],
                                 et[:, 0:512],
                                 start=(kt == 0), stop=(kt == NKT - 1))
                nc.tensor.matmul(av1[:], vh_aug[kt][:, 66:132],
                                 et[:, 512:1024],
                                 start=(kt == 0), stop=(kt == NKT - 1))
                if kt >= 4 and pending_norm and (kt - 4) < len(pending_norm):
                    pending_norm[kt - 4]()
                    if kt - 4 == len(pending_norm) - 1:
                        pending_norm = []
                if kt >= 8 and ci < len(next_chunks):
                    next_chunks[ci]()
                    ci += 1
            # evict av psum now (frees banks); defer transposes/output
            avss = []
            for hh, av in ((0, av0), (1, av1)):
                avs = avs_pool.tile([66, 512], F32, tag="avs")
                nc.vector.tensor_copy(avs[:], av[:])
                avss.append(avs)
            pending_norm = _make_norm(nc, tc, out_pool, psum_pp, rec_pool,
                                      out, id_sb, avss, b, qb)
        while ci < len(next_chunks):
            next_chunks[ci]()
            ci += 1
        cur = nxt
    for fn in pending_norm:
        fn()
    pending_norm = []


def _make_norm(nc, tc, out_pool, psum_pp, rec_pool, out, id_sb, avss, b, qb):
    """Build deferred per-(h,t) normalize+store steps for one qb."""
    ots = [out_pool.tile([128, 128], mybir.dt.float32, tag="ot",
                         name=f"ot{t}") for t in range(4)]
    steps = []

    def make_step(h, t):
        def step():
            avs = avss[h]
            tp = psum_pp.tile([128, 66], mybir.dt.float32, tag="pp",
                              name="tpn")
            nc.tensor.transpose(tp[:], avs[:, t * 128:(t + 1) * 128],
                                id_sb[0:66, 0:66])
            rec = rec_pool.tile([128, 1], mybir.dt.float32, tag="rec",
                                name="rec")
            nc.vector.reciprocal(rec[:], tp[:, 64:65])
            nc.vector.tensor_scalar_mul(
                ots[t][:, h * 64:(h + 1) * 64], tp[:, 0:64], rec[:])
            if h == 1:
                r0 = qb * 512 + t * 128
                nc.sync.dma_start(out[b, r0:r0 + 128, :], ots[t][:])
        return step

    for t in range(4):
        for h in (0, 1):
            steps.append(make_step(h, t))
    # merge per-(h,t) into per-slot pairs so 8 slots fit in 12 kt slots
    return [(lambda a=steps[i], c=steps[i + 1]: (a(), c()))
            for i in range(0, 8, 2)]


_CACHE = {}


def _build():
    if "nc" in _CACHE:
        return _CACHE["nc"]
    nc = bacc.Bacc("TRN2", target_bir_lowering=False, debug=False,
                   enable_asserts=False)
    qT = nc.dram_tensor("qT", [B, E, SQ], BF16, kind="ExternalInput").ap()
    vT = nc.dram_tensor("vT", [B, E, SK], F32R, kind="ExternalInput").ap()
    wq = nc.dram_tensor("wq", [E, CPC], BF16, kind="ExternalInput").ap()
    wk = nc.dram_tensor("wk", [E, CPC], F32R, kind="ExternalInput").ap()
    wv = nc.dram_tensor("wv", [E, CPC], F32R, kind="ExternalInput").ap()
    bq = nc.dram_tensor("bq", [CPC, 1], F32, kind="ExternalInput").ap()
    bk = nc.dram_tensor("bk", [CPC, 1], F32, kind="ExternalInput").ap()
    bv = nc.dram_tensor("bv", [CPC, 1], F32, kind="ExternalInput").ap()
    ident = nc.dram_tensor("ident", [128, 128], F32, kind="ExternalInput").ap()
    aug_const = nc.dram_tensor("aug_const", [128, 2], F32R,
                               kind="ExternalInput").ap()
    zeros = nc.dram_tensor("zeros", [64, SK], BF16,
                           kind="ExternalInput").ap()
    out = nc.dram_tensor("out", [B, SQ, CPC], F32, kind="ExternalOutput").ap()
    with tile.TileContext(nc) as tc:
        with ExitStack() as ctx:
            _body(ctx, tc, out, qT, vT, wq, wk, wv, bq, bk, bv, ident, aug_const,
                  zeros)
    nc.compile()
    _CACHE["nc"] = nc
    return nc


def _round_tf32(x):
    """Round-to-nearest-even to 10 explicit mantissa bits (TF32)."""
    u = np.ascontiguousarray(x, np.float32).view(np.uint32)
    r = (u + 0x1000 + ((u >> 13) & 1)) & np.uint32(0xFFFFE000)
    return r.view(np.float32)


def _in_maps(q, v, Wq, bq, Wk, bk, Wv, bv):
    f = np.float32
    import ml_dtypes
    qT = np.ascontiguousarray(
        np.transpose(np.asarray(q, f), (0, 2, 1))).astype(ml_dtypes.bfloat16)
    vT = _round_tf32(np.transpose(np.asarray(v, f), (0, 2, 1)))
    ident = np.eye(128, dtype=f)
    aug2 = np.tile(np.array([1.0, 0.0], f), (128, 1))
    zeros64 = np.zeros((64, SK), ml_dtypes.bfloat16)
    Wq, Wk, Wv = (np.asarray(x, f) for x in (Wq, Wk, Wv))
    bq, bk, bv = (np.asarray(x, f) for x in (bq, bk, bv))
    maps = []
    for c in range(NCORES):
        sl = slice(c * CPC, (c + 1) * CPC)
        maps.append({
            "qT": qT, "vT": vT, "ident": ident, "aug_const": aug2,
            "zeros": zeros64,
            "wq": np.ascontiguousarray(Wq[:, sl]).astype(ml_dtypes.bfloat16),
            "wk": _round_tf32(Wk[:, sl]),
            "wv": _round_tf32(Wv[:, sl]),
            "bq": np.ascontiguousarray(bq[sl]).reshape(CPC, 1),
            "bk": np.ascontiguousarray(bk[sl]).reshape(CPC, 1),
            "bv": np.ascontiguousarray(bv[sl]).reshape(CPC, 1),
        })
    return maps


def run(trace=False, **inputs):
    nc = _build()
    maps = _in_maps(**inputs)
    res = run_bass_kernel_spmd(nc, maps, core_ids=list(range(NCORES)),
                               trace=trace)
    full = np.concatenate([res.results[c]["out"] for c in range(NCORES)],
                          axis=2)
    return full, res


def kernel(q, v, Wq, bq, Wk, bk, Wv, bv):
    full, _ = run(q=q, v=v, Wq=Wq, bq=bq, Wk=Wk, bk=bk, Wv=Wv, bv=bv)
    return full
